# revision 27
# baseline (speedup 1.0000x reference)
"""Multi-head self-attention + vocab projection, 8-core TRN2 Bass kernel.

Problem: x[2,2048,1024] -> logits[2,2048,32000]
  q/k/v = x@W{q,k,v}+b, 16 heads x 64; attn = softmax(qk^T/8)v; out = attn@Wo+bo

Sharding: data-parallel over the 4096 token rows -> 8 cores x 512 query rows
(cores 0-3 batch 0, cores 4-7 batch 1). Each core receives its full batch
(2048 tokens) for K/V, ROLLED so that its 512 query rows are rows 0:512 —
softmax is permutation-invariant over the kv axis, so rolling is safe and
makes the SPMD program core-id independent. Wo is column-streamed in full on
every core; logits are written with no cross-core reduce.

x and all weights are converted to bf16 host-side (halves HBM traffic for
the dominant Wo stream; matmuls accumulate in f32 psum). xT comes from XBAR
DMA-transpose — no PE transposes. Attention is computed transposed
(scoresT[j,q] = kT^T qT) so exp(scoresT) feeds attn@V directly as lhsT and
the attention output lands as attn_outT[emb, tok] — exactly the lhsT layout
the vocab projection needs. The softmax denominator comes from an appended
ones-column on V; normalization is applied after attn@V via reciprocal +
DRAM-broadcast + elementwise multiply.

Projections and attention are INTERLEAVED per head-pair (the PE otherwise
micro-idles waiting on ACT exp between score/attn matmuls, which keeps the
HAM clock gate cold at 1.2 GHz — measured 300+ us of K=4/8 throttling in the
phase-separated version). Head-pair scores share one 2-bank psum tile so exp
runs once per kv-tile over [128, 1024]. All SBUF pools are top-level so Wo
prefetch DMAs can run during the attention phase.
"""

import numpy as np

B, S, E = 2, 2048, 1024
H, D = 16, 64
V = 32000
P = 128
ET = E // P          # 8 embedding tiles
TOK = S              # kv tokens per core
Q = 512              # query rows per core
NJT = TOK // P       # 16 kv tiles
VCH = 500            # vocab chunk (psum bank = 512 f32)
NVC = V // VCH       # 64
DVC = 256            # v-projection dout chunk (= 4 heads)
NCORES = 8

_cache = {}


def _build():
    from contextlib import ExitStack

    import concourse.tile as tile
    from concourse import bacc, mybir

    f32 = mybir.dt.float32
    bf16 = mybir.dt.bfloat16
    Id = mybir.ActivationFunctionType.Identity
    Exp = mybir.ActivationFunctionType.Exp

    nc = bacc.Bacc("TRN2", target_bir_lowering=False, debug=False,
                   num_devices=NCORES)

    xb = nc.dram_tensor("xb", [TOK, E], bf16, kind="ExternalInput").ap()
    wq = nc.dram_tensor("wq", [E, E], bf16, kind="ExternalInput").ap()
    wk = nc.dram_tensor("wk", [E, E], bf16, kind="ExternalInput").ap()
    wv = nc.dram_tensor("wv", [E, E], bf16, kind="ExternalInput").ap()
    wo = nc.dram_tensor("wo", [E, V], bf16, kind="ExternalInput").ap()
    bq = nc.dram_tensor("bq", [E], f32, kind="ExternalInput").ap()
    bk = nc.dram_tensor("bk", [E], f32, kind="ExternalInput").ap()
    bv = nc.dram_tensor("bv", [E], bf16, kind="ExternalInput").ap()
    bo = nc.dram_tensor("bo", [V], bf16, kind="ExternalInput").ap()
    out = nc.dram_tensor("out", [Q, V], bf16, kind="ExternalOutput").ap()

    wq3 = wq.rearrange("(et p) d -> p et d", p=P)
    wk3 = wk.rearrange("(et p) d -> p et d", p=P)
    wv3 = wv.rearrange("(et p) d -> p et d", p=P)
    wo3 = wo.rearrange("(et p) v -> p et v", p=P)

    with tile.TileContext(nc) as tc, ExitStack() as ctx:
        # ---- pools (all top-level: scheduling is purely dep-driven) ----
        consts = ctx.enter_context(tc.tile_pool(name="consts", bufs=1))
        xT_pool = ctx.enter_context(tc.tile_pool(name="xT", bufs=1))
        kT_pool = ctx.enter_context(tc.tile_pool(name="kT", bufs=1))
        vA_pool = ctx.enter_context(tc.tile_pool(name="vA", bufs=1))
        qT_pool = ctx.enter_context(tc.tile_pool(name="qT", bufs=1))
        aT_pool = ctx.enter_context(tc.tile_pool(name="aT", bufs=1))
        dn_pool = ctx.enter_context(tc.tile_pool(name="dn", bufs=1))
        wqk_pool = ctx.enter_context(tc.tile_pool(name="wqk", bufs=3))
        wv_pool = ctx.enter_context(tc.tile_pool(name="wvp", bufs=2))
        e_pool = ctx.enter_context(tc.tile_pool(name="epool", bufs=4))
        den_pool = ctx.enter_context(tc.tile_pool(name="denrow", bufs=2))
        den2_pool = ctx.enter_context(tc.tile_pool(name="den2", bufs=1))
        rbc_pool = ctx.enter_context(tc.tile_pool(name="rbc", bufs=2))
        wo_pool = ctx.enter_context(tc.tile_pool(name="wo", bufs=5))
        lt_pool = ctx.enter_context(tc.tile_pool(name="lt", bufs=4))
        bo_pool = ctx.enter_context(tc.tile_pool(name="bo", bufs=3))
        dram_pool = ctx.enter_context(
            tc.tile_pool(name="dramscratch", bufs=1, space="DRAM"))
        # PSUM: shared 2-bank-slot pool (3 bufs) + attention accums (2x1
        # bank) = 8 banks exactly
        psP = ctx.enter_context(tc.tile_pool(name="ps", bufs=3,
                                             space="PSUM"))
        psA = ctx.enter_context(tc.tile_pool(name="psA", bufs=2,
                                             space="PSUM"))

        bq_sb = consts.tile([P, ET], f32)
        nc.sync.dma_start(bq_sb[:], bq.rearrange("(g p) -> p g", p=P))
        # fold the 1/sqrt(d) score scale into q: q' = (xWq + bq)/8
        nc.vector.tensor_scalar_mul(bq_sb[:], bq_sb[:], 0.125)
        bk_sb = consts.tile([P, ET], f32)
        nc.sync.dma_start(bk_sb[:], bk.rearrange("(g p) -> p g", p=P))
        bv_bc = consts.tile([P, E], bf16)
        nc.sync.dma_start(
            bv_bc[:],
            bv.rearrange("(o e) -> o e", o=1).to_broadcast((P, E)))

        xT = xT_pool.tile([P, ET, TOK], bf16)    # x^T, emb on partitions
        kT = kT_pool.tile([P, ET, TOK], bf16)    # k^T, d on partitions
        vA = vA_pool.tile([P, NJT, H * 65], bf16)  # v + ones col, per kv tile
        qT = qT_pool.tile([P, ET, Q], bf16)      # (q/8)^T
        aT = aT_pool.tile([P, ET, Q], bf16)      # attn_out^T
        den_dram = dram_pool.tile([H, Q], f32)
        rec_dram = dram_pool.tile([H, Q], f32)

        # ones column of vA (head-local column 64); on GpSimd so the DVE
        # queue stays clear for the first projection-bias drains
        vA4 = vA[:].rearrange("p j (h c) -> p j h c", c=65)
        nc.gpsimd.memset(vA4[:, :, :, 64:65], 1.0)

        # prefetch the first weight tiles BEFORE the transposes so the
        # first projection matmuls aren't queued behind them
        wk0 = wqk_pool.tile([P, ET, P], bf16, tag="wqk")
        nc.sync.dma_start(wk0[:], wk3[:, :, 0:P])
        wv0 = wv_pool.tile([P, ET, DVC], bf16, tag="wv")
        nc.scalar.dma_start(wv0[:], wv3[:, :, 0:DVC])

        # x^T via XBAR dma-transpose, split across both HWDGE queues.
        # (Do NOT split these per token range: a partial-width transpose
        # destination produces wrong data on hardware — known xbar issue.)
        for et in range(ET):
            eng = nc.sync if et % 2 == 0 else nc.scalar
            eng.dma_start_transpose(xT[:, et, :],
                                    xb[:, et * P:(et + 1) * P])

        # ---- interleaved projections + attention -----------------------
        def proj_kq(g, wk_t, wq_t):
            # kT[:, g, :] (dout tile g = heads 2g, 2g+1)
            for tcc in range(TOK // 512):
                ps = psP.tile([P, 1024], f32, tag="ps")
                for et in range(ET):
                    nc.tensor.matmul(ps[:, 0:512], wk_t[:, et, :],
                                     xT[:, et, tcc * 512:(tcc + 1) * 512],
                                     start=(et == 0), stop=(et == ET - 1))
                nc.vector.tensor_scalar_add(
                    kT[:, g, tcc * 512:(tcc + 1) * 512], ps[:, 0:512],
                    bk_sb[:, g:g + 1])
            # qT[:, g, :] (score scale 1/8 folded in)
            ps = psP.tile([P, 1024], f32, tag="ps")
            for et in range(ET):
                nc.tensor.matmul(ps[:, 0:512], wq_t[:, et, :],
                                 xT[:, et, 0:Q],
                                 start=(et == 0), stop=(et == ET - 1))
            nc.vector.tensor_scalar(qT[:, g, :], ps[:, 0:512], 0.125,
                                    bq_sb[:, g:g + 1],
                                    mybir.AluOpType.mult,
                                    mybir.AluOpType.add)

        def proj_v(dvc, wv_t):
            for tt in range(NJT):
                ps = psP.tile([P, 1024], f32, tag="ps")
                psv = ps[:, 0:DVC]
                for et in range(ET):
                    nc.tensor.matmul(psv,
                                     xT[:, et, tt * P:(tt + 1) * P],
                                     wv_t[:, et, :],
                                     start=(et == 0), stop=(et == ET - 1))
                h0 = dvc * (DVC // D)
                dst = vA4[:, tt, h0:h0 + DVC // D, 0:64]
                bvs = bv_bc[:, dvc * DVC:(dvc + 1) * DVC]
                nc.vector.tensor_tensor(
                    dst, psv.rearrange("p (h c) -> p h c", c=D),
                    bvs.rearrange("p (h c) -> p h c", c=D),
                    mybir.AluOpType.add)

        def load_wk(g):
            t = wqk_pool.tile([P, ET, P], bf16, tag="wqk")
            nc.sync.dma_start(t[:], wk3[:, :, g * P:(g + 1) * P])
            return t

        def load_wq(g):
            t = wqk_pool.tile([P, ET, P], bf16, tag="wqk")
            nc.sync.dma_start(t[:], wq3[:, :, g * P:(g + 1) * P])
            return t

        def load_wv(dvc):
            t = wv_pool.tile([P, ET, DVC], bf16, tag="wv")
            nc.sync.dma_start(t[:], wv3[:, :, dvc * DVC:(dvc + 1) * DVC])
            return t

        # prologue: pair 0 + v chunk 0, then run one pair of projections
        # AHEAD of the attention that consumes them — the projection psum
        # drains (DVE) then complete during the previous pair's attention
        # instead of stalling the scores.
        proj_kq(0, wk0, load_wq(0))
        proj_v(0, wv0)
        for g in range(ET):
            if g + 1 < ET:
                proj_kq(g + 1, load_wk(g + 1), load_wq(g + 1))
                if (g + 1) % 2 == 0:
                    proj_v((g + 1) // 2, load_wv((g + 1) // 2))

            # attention for heads 2g (rows 0:64) and 2g+1 (rows 64:128);
            # the two K=64 score matmuls pack into array row-halves via
            # tile_position and run concurrently.
            h0, h1 = 2 * g, 2 * g + 1
            po0 = psA.tile([P, Q], f32, tag="a")
            po1 = psA.tile([P, Q], f32, tag="a")
            for jt in range(NJT):
                ps = psP.tile([P, 2 * Q], f32, tag="ps")
                nc.tensor.matmul(ps[:, 0:Q],
                                 kT[0:D, g, jt * P:(jt + 1) * P],
                                 qT[0:D, g, :], start=True, stop=True,
                                 tile_position=(0, 0))
                nc.tensor.matmul(ps[:, Q:2 * Q],
                                 kT[D:P, g, jt * P:(jt + 1) * P],
                                 qT[D:P, g, :], start=True, stop=True,
                                 tile_position=(64, 0))
                e = e_pool.tile([P, 2 * Q], bf16, tag="e")
                nc.scalar.activation(e[:], ps[:], Exp)
                nc.tensor.matmul(po0[0:65, :],
                                 vA[:, jt, h0 * 65:h0 * 65 + 65],
                                 e[:, 0:Q],
                                 start=(jt == 0), stop=(jt == NJT - 1))
                nc.tensor.matmul(po1[0:65, :],
                                 vA[:, jt, h1 * 65:h1 * 65 + 65],
                                 e[:, Q:2 * Q],
                                 start=(jt == 0), stop=(jt == NJT - 1))
            # denom rows live on psum partition 64; engines can't move
            # across partitions, so bounce via SBUF row 64 + DMA. These
            # drains run on DVE: on ACT they'd queue behind the exps and
            # delay the psum-accumulator release for the next pair.
            for po, hh, dr in ((po0, h0, 0), (po1, h1, D)):
                den_t = den_pool.tile([P, Q], f32, tag="denrow")
                nc.vector.tensor_copy(den_t[64:65, :], po[64:65, :])
                nc.sync.dma_start(den_dram[hh:hh + 1, :], den_t[64:65, :])
                nc.vector.tensor_copy(aT[dr:dr + D, g, :], po[0:64, :])

            # normalize this pair by its softmax denominators (per-pair so
            # the chain overlaps later pairs' attention instead of
            # serializing at the end)
            den2 = den2_pool.tile([2, Q], f32, tag="den2")
            nc.sync.dma_start(den2[:], den_dram[h0:h1 + 1, :])
            rec2 = den2_pool.tile([2, Q], f32, tag="rec2")
            nc.vector.reciprocal(rec2[:], den2[:])
            nc.sync.dma_start(rec_dram[h0:h1 + 1, :], rec2[:])
            rbc = rbc_pool.tile([P, Q], f32, tag="rbc")
            nc.sync.dma_start(
                rbc[0:D, :], rec_dram[h0:h0 + 1, :].to_broadcast((D, Q)))
            nc.sync.dma_start(
                rbc[D:P, :], rec_dram[h1:h1 + 1, :].to_broadcast((D, Q)))
            nc.vector.tensor_tensor(aT[:, g, :], aT[:, g, :], rbc[:],
                                    mybir.AluOpType.mult)

        # ---- vocab projection ------------------------------------------
        for vc in range(NVC):
            wo_t = wo_pool.tile([P, ET, VCH], bf16, tag="wo")
            nc.scalar.dma_start(wo_t[:], wo3[:, :, vc * VCH:(vc + 1) * VCH])
            bo_t = bo_pool.tile([P, VCH], bf16, tag="bo")
            nc.sync.dma_start(
                bo_t[:],
                bo[vc * VCH:(vc + 1) * VCH]
                .rearrange("(o v) -> o v", o=1).to_broadcast((P, VCH)))
            for tc4 in range(Q // P):
                ps = psP.tile([P, 1024], f32, tag="ps")
                pso = ps[:, 0:VCH]
                for et in range(ET):
                    nc.tensor.matmul(pso,
                                     aT[:, et, tc4 * P:(tc4 + 1) * P],
                                     wo_t[:, et, :],
                                     start=(et == 0), stop=(et == ET - 1))
                lt = lt_pool.tile([P, VCH], bf16, tag="lt")
                nc.vector.tensor_tensor(lt[:], pso, bo_t[:],
                                        mybir.AluOpType.add)
                nc.scalar.dma_start(
                    out[tc4 * P:(tc4 + 1) * P, vc * VCH:(vc + 1) * VCH],
                    lt[:])

    nc.compile()
    return nc


def get_nc():
    if "nc" not in _cache:
        _cache["nc"] = _build()
    return _cache["nc"]


def make_in_maps(x, Wq, bq, Wk, bk, Wv, bv, Wo, bo):
    import ml_dtypes

    def bf(a):
        return np.asarray(np.asarray(a, dtype=np.float32)
                          .astype(ml_dtypes.bfloat16))

    def f32a(a):
        return np.ascontiguousarray(np.asarray(a, dtype=np.float32))

    x = bf(x)
    Wq, Wk, Wv, Wo = bf(Wq), bf(Wk), bf(Wv), bf(Wo)
    bv, bo = bf(bv), bf(bo)
    bq, bk = f32a(bq), f32a(bk)
    in_maps = []
    for c in range(NCORES):
        b, qs = c // 4, (c % 4) * Q
        xbm = np.ascontiguousarray(np.roll(x[b], -qs, axis=0))
        in_maps.append({"xb": xbm, "wq": Wq, "wk": Wk, "wv": Wv, "wo": Wo,
                        "bq": bq, "bk": bk, "bv": bv, "bo": bo})
    return in_maps


def gather(results):
    out = np.empty((B, S, V), dtype=np.float32)
    for c in range(NCORES):
        b, qs = c // 4, (c % 4) * Q
        out[b, qs:qs + Q] = np.asarray(results[c]["out"],
                                       dtype=np.float32)
    return out


def kernel(**inputs):
    from concourse.bass_utils import run_bass_kernel_spmd

    nc = get_nc()
    in_maps = make_in_maps(**inputs)
    res = run_bass_kernel_spmd(nc, in_maps, list(range(NCORES)), trace=False)
    return gather(res.results)
